# revision 21
# baseline (speedup 1.0000x reference)
"""Trainium2 Bass kernel for nn_LuenbergerLDS (B=32, T=2048, N=512, M=512).

Math: the reference is a diagonal complex linear recurrence
    s_t = lam * s_{t-1} + x_t   (per batch, per n; x scalar per t broadcast over n)
followed by  y = Re(Winv @ s) @ C + x @ D + Do.

Since d == 1 the whole module is a causal LTI SIMO filter:
    y[t, b, m] = sum_{j>=0} H[j, m] * x[t - j, b] + Do[m]
with impulse response (computed on host in float64)
    H[j, m] = sum_n Re(lam_n^j) * A_re[n, m] - Im(lam_n^j) * A_im[n, m]
    A_re = Re(Winv)^T @ C,  A_im = Im(Winv)^T @ C,  H[0] += D.
All modes decay at >= 0.012/step, so a window of NLAG*128 = 1024 lags
truncates at < 6e-7 of max|y| (measured exactly on the reference data).

Device work (per core, data-parallel over batch: 4 batches/core): pure
matmuls. For output chunk t0..t0+127 and lag tile `lag`, the 128x128
stationary operand is a (Toeplitz) diagonal slice of a lag-shifted x
buffer xsh in SBUF, the moving operand is a row-flipped H tile
(128x512), accumulated in one PSUM bank over all lag tiles.
xsh[p, u, b] = xpad[u + p, b] is materialized by strided (diagonal)
DMAs from a zero-padded DRAM copy of x.

dtype: float32r (PE processes it 4x faster than float32). f32r matmul
is EXACT for operands with <= 12 explicit mantissa bits (HW-probed), so
operands are pre-rounded on host to that grid, and the dominant head
lag tile (lags 0..127) gets two extra Dekker-compensation passes
(x_hi*H_lo + x_lo*H_hi), making the head exact to fp32 quality. The
tail's single-pass quantization error lands at ~1e-5 of max|y|
(simulated on the reference data).
"""

import os
import sys

sys.path.insert(0, "/opt/trn_rl_repo")

import numpy as np

# problem dims (hardcoded per harness contract)
B, T, N, M = 32, 2048, 512, 512
NCORES = 8
BLOC = B // NCORES          # batches per core
NLAG = int(os.environ.get("K_NLAG", "6"))
MODE = os.environ.get("K_MODE", "f32r_hybrid")  # f32 | f32r1 | f32r_hybrid | f32r3


def _dims(t, nlag, bloc):
    rpad = 128 * nlag - 1
    u = t + 128 * nlag - 128
    tch = t // 128
    return rpad, u, tch


def build_program(t=T, m=M, nlag=NLAG, bloc=BLOC, nseg=8, mode=MODE):
    """Build + compile the (SPMD, per-core) Bass program."""
    import concourse.tile as tile
    from concourse import bacc, mybir
    from bass_rust import VecI64Pair

    rpad, u, tch = _dims(t, nlag, bloc)
    f32 = mybir.dt.float32
    f32r = mybir.dt.float32r
    bf16 = mybir.dt.bfloat16
    mm_dt = f32 if mode == "f32" else f32r
    need_lo = mode in ("f32r_hybrid", "f32r3")
    nlo = nlag if mode == "f32r3" else (1 if mode == "f32r_hybrid" else 0)
    # NOTE: running the lo-compensation matmuls in bf16 was tried and costs
    # more than it saves: mixing dtypes in the MM stream slowed the median
    # MM issue gap 237->285ns (HW-measured). Keep every matmul f32r.
    lo_dt = mm_dt

    nc = bacc.Bacc("TRN2", target_bir_lowering=False, debug=False)
    xpad_t = nc.dram_tensor("xpad", [rpad + t, bloc], mm_dt, kind="ExternalInput")
    ht_t = nc.dram_tensor("ht", [nlag * 128, m], mm_dt, kind="ExternalInput")
    if need_lo:
        xpadlo_t = nc.dram_tensor(
            "xpadlo", [rpad + t, bloc], lo_dt, kind="ExternalInput"
        )
        htlo_t = nc.dram_tensor("htlo", [nlo * 128, m], mm_dt, kind="ExternalInput")
    dorep_t = nc.dram_tensor("dorep", [128, m], f32, kind="ExternalInput")
    y_t = nc.dram_tensor("y", [bloc, t, m], f32, kind="ExternalOutput")

    n_d = u // 128                  # number of 128-wide u-slices ("diagonals")
    d_lo0 = 0 if mode == "f32r3" else nlag - 1  # first diagonal the lo pass reads

    with tile.TileContext(nc) as tc:
        with (
            tc.tile_pool(name="xsh", bufs=1) as xsh_pool,
            tc.tile_pool(name="w", bufs=1) as wpool,
            tc.tile_pool(name="psum", bufs=8, space="PSUM") as psum_pool,
            tc.tile_pool(name="out", bufs=4) as out_pool,
        ):
            # Slices with d <= nlag-3 read only the zero padding of xpad:
            # skip their loads AND their matmuls entirely.
            d_hi0 = max(0, nlag - 3 + 1)

            # DMA plan: the first groups' inputs (all H tiles + first hi/lo
            # slices) spread across all three DMA queues (each queue only
            # sustains ~1/3 of fabric bandwidth), each queue ordered
            # critical-first; bulk slices appended behind.
            def load_slice(pool, dram_t, d, tag, dt_, eng):
                tl = pool.tile([128, 128 * bloc], dt_, tag=tag)
                in_ap = dram_t.ap().copy()
                in_ap.ap = VecI64Pair([[bloc, 128], [bloc, 128], [1, bloc]])
                in_ap.offset = d * 128 * bloc
                eng.dma_start(out=tl[:], in_=in_ap)
                return tl[:].rearrange("p (uu b) -> p uu b", b=bloc)

            ht_sb = [None] * nlag
            htlo_sb = [None] * nlo
            hi_sl = {}
            lo_sl = {}
            do_sb = None

            # build the load list: (kind, index) in critical-first order
            loads = []
            seen = set()

            def add(kind, idx):
                if (kind, idx) not in seen:
                    seen.add((kind, idx))
                    loads.append((kind, idx))

            for lg in range(nlag):
                add("ht", lg)
            for lg in range(nlo):
                add("htlo", lg)
            add("dorep", 0)
            add("hi", nlag - 1)
            if nlag - 2 >= d_hi0:
                add("hi", nlag - 2)
            if need_lo:
                add("lo", max(d_lo0, nlag - 1))
            for d in range(d_hi0, n_d):
                add("hi", d)
                if need_lo and d >= d_lo0:
                    add("lo", d)

            engines = [nc.sync, nc.scalar, nc.gpsimd]
            for i, (kind, idx) in enumerate(loads):
                eng = engines[i % 3]
                if kind == "ht":
                    w_tile = wpool.tile([128, m], mm_dt, tag=f"ht{idx}")
                    eng.dma_start(
                        w_tile[:], ht_t.ap()[idx * 128 : (idx + 1) * 128, :]
                    )
                    ht_sb[idx] = w_tile
                elif kind == "htlo":
                    w_tile = wpool.tile([128, m], mm_dt, tag=f"htlo{idx}")
                    eng.dma_start(
                        w_tile[:], htlo_t.ap()[idx * 128 : (idx + 1) * 128, :]
                    )
                    htlo_sb[idx] = w_tile
                elif kind == "dorep":
                    do_sb = wpool.tile([128, m], f32, tag="dorep")
                    eng.dma_start(do_sb[:], dorep_t.ap())
                elif kind == "hi":
                    hi_sl[idx] = load_slice(
                        xsh_pool, xpad_t, idx, f"hi{idx}", mm_dt, eng
                    )
                else:
                    lo_sl[idx] = load_slice(
                        xsh_pool, xpadlo_t, idx, f"lo{idx}", lo_dt, eng
                    )

            # tc-outer / b-inner: the 4 batches' groups for one output chunk
            # reuse the same x slices, so the input DMA stream feeds 4x more
            # compute (keeps the fabric under its bandwidth during chunk 0)
            for tci in range(tch):
                for b in range(bloc):
                    # accumulation group: (stationary, moving) pairs; skip
                    # matmuls whose stationary slice is all zero padding
                    mms = []
                    for lg in range(nlag):
                        d = tci - lg + nlag - 1
                        if d >= d_hi0:
                            mms.append((hi_sl[d][:, :, b], ht_sb[lg]))
                    for lg in range(nlo):
                        d = tci - lg + nlag - 1
                        if d >= d_hi0:
                            mms.append((hi_sl[d][:, :, b], htlo_sb[lg]))
                            mms.append((lo_sl[d][:, :, b], ht_sb[lg]))
                    ps = psum_pool.tile([128, m], f32)
                    for i, (lhs, rhs) in enumerate(mms):
                        nc.tensor.matmul(
                            ps[:],
                            lhsT=lhs,
                            rhs=rhs[:],
                            start=(i == 0),
                            stop=(i == len(mms) - 1),
                        )
                    ot = out_pool.tile([128, m], f32)
                    nc.vector.tensor_add(ot[:], ps[:], do_sb[:])
                    nc.sync.dma_start(
                        y_t.ap()[b, 128 * tci : 128 * tci + 128, :], ot[:]
                    )

    nc.compile()
    return nc


def _round_mant(a, bits=12):
    """Round float64 array to `bits` explicit mantissa bits (RNE)."""
    m, e = np.frexp(a)
    s = 2.0 ** bits
    return np.round(m * s) / s * 2.0 ** e


def host_weights(lnl_re, lnl_im, W_r, W_i, C, D, Do, t=T, m=M, nlag=NLAG, mode=MODE):
    """Impulse response H (flipped per 128-tile) + replicated Do, float64 math."""
    lnl = lnl_re.astype(np.float64) + 1j * lnl_im.astype(np.float64)
    W = W_r.astype(np.float64) + 1j * W_i.astype(np.float64)
    Winv = np.linalg.inv(W)
    A_re = np.ascontiguousarray(Winv.real.T) @ C.astype(np.float64)
    A_im = np.ascontiguousarray(Winv.imag.T) @ C.astype(np.float64)
    j = np.arange(nlag * 128, dtype=np.float64)
    P = np.exp(np.outer(j, lnl))                      # lam^j, (W, N) complex128
    H = P.real @ A_re - P.imag @ A_im                 # (W, M)
    H[0] += D[0].astype(np.float64)

    def flip_tiles(Hm, ntile):
        Hf = Hm.reshape(ntile, 128, m)[:, ::-1, :]
        return np.ascontiguousarray(Hf.reshape(ntile * 128, m)).astype(np.float32)

    dorep = np.ascontiguousarray(np.broadcast_to(Do.astype(np.float32), (128, m)))
    if mode == "f32":
        return {"ht": flip_tiles(H, nlag), "dorep": dorep}
    H_hi = _round_mant(H)
    if mode == "f32r1":
        return {"ht": flip_tiles(H_hi, nlag), "dorep": dorep}
    nlo = nlag if mode == "f32r3" else 1
    H_lo = _round_mant(H[: nlo * 128] - H_hi[: nlo * 128])
    return {
        "ht": flip_tiles(H_hi, nlag),
        "htlo": flip_tiles(H_lo, nlo),
        "dorep": dorep,
    }


def make_in_maps(x, weights, t=T, nlag=NLAG, bloc=BLOC, ncores=NCORES, mode=MODE):
    rpad, _, _ = _dims(t, nlag, bloc)
    lo_np = np.float32
    x64 = x[:, :, 0].astype(np.float64)
    if mode == "f32":
        x_hi, x_lo = x64, None
    else:
        x_hi = _round_mant(x64)
        x_lo = _round_mant(x64 - x_hi) if mode in ("f32r_hybrid", "f32r3") else None
    in_maps = []
    for c in range(ncores):
        sl = slice(c * bloc, (c + 1) * bloc)
        xpad = np.zeros((rpad + t, bloc), np.float32)
        xpad[rpad:, :] = x_hi[sl].T
        im = dict(weights)
        im["xpad"] = xpad
        if x_lo is not None:
            xpadlo = np.zeros((rpad + t, bloc), lo_np)
            xpadlo[rpad:, :] = x_lo[sl].T.astype(lo_np)
            im["xpadlo"] = xpadlo
        in_maps.append(im)
    return in_maps


_prog_cache = {}


def kernel(x, lnl_re, lnl_im, W_r, W_i, C, D, Do):
    from concourse.bass_utils import run_bass_kernel_spmd

    # coerce to numpy (host math needs real float64; jax arrays stay fp32)
    x = np.asarray(x)
    lnl_re, lnl_im = np.asarray(lnl_re), np.asarray(lnl_im)
    W_r, W_i = np.asarray(W_r), np.asarray(W_i)
    C, D, Do = np.asarray(C), np.asarray(D), np.asarray(Do)

    key = (NLAG, MODE)
    if key not in _prog_cache:
        _prog_cache[key] = build_program()
    nc = _prog_cache[key]

    weights = host_weights(lnl_re, lnl_im, W_r, W_i, C, D, Do)
    in_maps = make_in_maps(np.asarray(x, np.float32), weights)
    res = run_bass_kernel_spmd(nc, in_maps, core_ids=list(range(NCORES)))
    y = np.concatenate([res.results[i]["y"] for i in range(NCORES)], axis=0)
    return np.ascontiguousarray(y.astype(np.float32))


# revision 24
# speedup vs baseline: 1.0796x; 1.0796x over previous
"""Trainium2 Bass kernel for nn_LuenbergerLDS (B=32, T=2048, N=512, M=512).

Math: the reference is a diagonal complex linear recurrence
    s_t = lam * s_{t-1} + x_t   (per batch, per n; x scalar per t broadcast over n)
followed by  y = Re(Winv @ s) @ C + x @ D + Do.

Since d == 1 the whole module is a causal LTI SIMO filter:
    y[t, b, m] = sum_{j>=0} H[j, m] * x[t - j, b] + Do[m]
with impulse response (computed on host in float64)
    H[j, m] = sum_n Re(lam_n^j) * A_re[n, m] - Im(lam_n^j) * A_im[n, m]
    A_re = Re(Winv)^T @ C,  A_im = Im(Winv)^T @ C,  H[0] += D.
All modes decay at >= 0.012/step, so a window of NLAG*128 = 1024 lags
truncates at < 6e-7 of max|y| (measured exactly on the reference data).

Device work (per core, data-parallel over batch: 4 batches/core): pure
matmuls. For output chunk t0..t0+127 and lag tile `lag`, the 128x128
stationary operand is a (Toeplitz) diagonal slice of a lag-shifted x
buffer xsh in SBUF, the moving operand is a row-flipped H tile
(128x512), accumulated in one PSUM bank over all lag tiles.
xsh[p, u, b] = xpad[u + p, b] is materialized by strided (diagonal)
DMAs from a zero-padded DRAM copy of x.

dtype: float32r (PE processes it 4x faster than float32). f32r matmul
is EXACT for operands with <= 12 explicit mantissa bits (HW-probed), so
operands are pre-rounded on host to that grid, and the dominant head
lag tile (lags 0..127) gets two extra Dekker-compensation passes
(x_hi*H_lo + x_lo*H_hi), making the head exact to fp32 quality. The
tail's single-pass quantization error lands at ~1e-5 of max|y|
(simulated on the reference data).
"""

import os
import sys

sys.path.insert(0, "/opt/trn_rl_repo")

import numpy as np

# problem dims (hardcoded per harness contract)
B, T, N, M = 32, 2048, 512, 512
NCORES = 8
BLOC = B // NCORES          # batches per core
NLAG = int(os.environ.get("K_NLAG", "6"))
MODE = os.environ.get("K_MODE", "f32r_hybrid")  # f32 | f32r1 | f32r_hybrid | f32r3


def _dims(t, nlag, bloc):
    rpad = 128 * nlag - 1
    u = t + 128 * nlag - 128
    tch = t // 128
    return rpad, u, tch


def build_program(t=T, m=M, nlag=NLAG, bloc=BLOC, nseg=8, mode=MODE):
    """Build + compile the (SPMD, per-core) Bass program."""
    import concourse.tile as tile
    from concourse import bacc, mybir
    from bass_rust import VecI64Pair

    rpad, u, tch = _dims(t, nlag, bloc)
    f32 = mybir.dt.float32
    f32r = mybir.dt.float32r
    bf16 = mybir.dt.bfloat16
    mm_dt = f32 if mode == "f32" else f32r
    need_lo = mode in ("f32r_hybrid", "f32r3")
    nlo = nlag if mode == "f32r3" else (1 if mode == "f32r_hybrid" else 0)
    # NOTE: running the lo-compensation matmuls in bf16 was tried and costs
    # more than it saves: mixing dtypes in the MM stream slowed the median
    # MM issue gap 237->285ns (HW-measured). Keep every matmul f32r.
    lo_dt = mm_dt

    nc = bacc.Bacc("TRN2", target_bir_lowering=False, debug=False)
    xpad_t = nc.dram_tensor("xpad", [rpad + t, bloc], mm_dt, kind="ExternalInput")
    ht_t = nc.dram_tensor("ht", [nlag * 128, m], mm_dt, kind="ExternalInput")
    if need_lo:
        xpadlo_t = nc.dram_tensor(
            "xpadlo", [rpad + t, bloc], lo_dt, kind="ExternalInput"
        )
        htlo_t = nc.dram_tensor("htlo", [nlo * 128, m], mm_dt, kind="ExternalInput")
    dorep_t = nc.dram_tensor("dorep", [128, m], f32, kind="ExternalInput")
    y_t = nc.dram_tensor("y", [bloc, t, m], f32, kind="ExternalOutput")

    n_d = u // 128                  # number of 128-wide u-slices ("diagonals")
    d_lo0 = 0 if mode == "f32r3" else nlag - 1  # first diagonal the lo pass reads

    with tile.TileContext(nc) as tc:
        with (
            tc.tile_pool(name="xsh", bufs=1) as xsh_pool,
            tc.tile_pool(name="w", bufs=1) as wpool,
            tc.tile_pool(name="psum", bufs=8, space="PSUM") as psum_pool,
            tc.tile_pool(name="out", bufs=4) as out_pool,
        ):
            # Slice d spans xpad rows [128d, 128d+254]; rows < rpad=128*nlag-1
            # are zero padding, so slices with d <= nlag-2 are entirely zero:
            # skip their loads AND their matmuls.
            d_hi0 = max(0, nlag - 1)

            # DMA plan: the first groups' inputs (all H tiles + first hi/lo
            # slices) spread across all three DMA queues (each queue only
            # sustains ~1/3 of fabric bandwidth), each queue ordered
            # critical-first; bulk slices appended behind.
            def load_slice(pool, dram_t, d, tag, dt_, eng):
                tl = pool.tile([128, 128 * bloc], dt_, tag=tag)
                in_ap = dram_t.ap().copy()
                in_ap.ap = VecI64Pair([[bloc, 128], [bloc, 128], [1, bloc]])
                in_ap.offset = d * 128 * bloc
                eng.dma_start(out=tl[:], in_=in_ap)
                return tl[:].rearrange("p (uu b) -> p uu b", b=bloc)

            ht_sb = [None] * nlag
            htlo_sb = [None] * nlo
            hi_sl = {}
            lo_sl = {}
            do_sb = None

            # build the load list: (kind, index) in critical-first order
            loads = []
            seen = set()

            def add(kind, idx):
                if (kind, idx) not in seen:
                    seen.add((kind, idx))
                    loads.append((kind, idx))

            # first-use order: group tc first touches ht[tc], hi[tc+nlag-1],
            # lo[tc+nlag-1] (while tc < nlag); dorep is needed at the first
            # evacuation, after group 0's matmuls
            for tc_ in range(nlag):
                add("ht", tc_)
                if tc_ < nlo:
                    add("htlo", tc_)
                d = tc_ + nlag - 1
                if d < n_d:
                    add("hi", d)
                    if need_lo and d >= d_lo0:
                        add("lo", d)
                if tc_ == 1:
                    add("dorep", 0)
            for lg in range(nlo):
                add("htlo", lg)
            add("dorep", 0)
            for d in range(d_hi0, n_d):
                add("hi", d)
                if need_lo and d >= d_lo0:
                    add("lo", d)

            engines = [nc.sync, nc.scalar, nc.gpsimd]
            for i, (kind, idx) in enumerate(loads):
                eng = engines[i % 3]
                if kind == "ht":
                    w_tile = wpool.tile([128, m], mm_dt, tag=f"ht{idx}")
                    eng.dma_start(
                        w_tile[:], ht_t.ap()[idx * 128 : (idx + 1) * 128, :]
                    )
                    ht_sb[idx] = w_tile
                elif kind == "htlo":
                    w_tile = wpool.tile([128, m], mm_dt, tag=f"htlo{idx}")
                    eng.dma_start(
                        w_tile[:], htlo_t.ap()[idx * 128 : (idx + 1) * 128, :]
                    )
                    htlo_sb[idx] = w_tile
                elif kind == "dorep":
                    do_sb = wpool.tile([128, m], f32, tag="dorep")
                    eng.dma_start(do_sb[:], dorep_t.ap())
                elif kind == "hi":
                    hi_sl[idx] = load_slice(
                        xsh_pool, xpad_t, idx, f"hi{idx}", mm_dt, eng
                    )
                else:
                    lo_sl[idx] = load_slice(
                        xsh_pool, xpadlo_t, idx, f"lo{idx}", lo_dt, eng
                    )

            for b in range(bloc):
                for tci in range(tch):
                    # accumulation group: (stationary, moving) pairs; skip
                    # matmuls whose stationary slice is all zero padding
                    mms = []
                    for lg in range(nlag):
                        d = tci - lg + nlag - 1
                        if d >= d_hi0:
                            mms.append((hi_sl[d][:, :, b], ht_sb[lg]))
                    for lg in range(nlo):
                        d = tci - lg + nlag - 1
                        if d >= d_hi0:
                            mms.append((hi_sl[d][:, :, b], htlo_sb[lg]))
                            mms.append((lo_sl[d][:, :, b], ht_sb[lg]))
                    ps = psum_pool.tile([128, m], f32)
                    for i, (lhs, rhs) in enumerate(mms):
                        nc.tensor.matmul(
                            ps[:],
                            lhsT=lhs,
                            rhs=rhs[:],
                            start=(i == 0),
                            stop=(i == len(mms) - 1),
                        )
                    ot = out_pool.tile([128, m], f32)
                    nc.vector.tensor_add(ot[:], ps[:], do_sb[:])
                    nc.sync.dma_start(
                        y_t.ap()[b, 128 * tci : 128 * tci + 128, :], ot[:]
                    )

    nc.compile()
    return nc


def _round_mant(a, bits=12):
    """Round float64 array to `bits` explicit mantissa bits (RNE)."""
    m, e = np.frexp(a)
    s = 2.0 ** bits
    return np.round(m * s) / s * 2.0 ** e


def host_weights(lnl_re, lnl_im, W_r, W_i, C, D, Do, t=T, m=M, nlag=NLAG, mode=MODE):
    """Impulse response H (flipped per 128-tile) + replicated Do, float64 math."""
    lnl = lnl_re.astype(np.float64) + 1j * lnl_im.astype(np.float64)
    W = W_r.astype(np.float64) + 1j * W_i.astype(np.float64)
    Winv = np.linalg.inv(W)
    A_re = np.ascontiguousarray(Winv.real.T) @ C.astype(np.float64)
    A_im = np.ascontiguousarray(Winv.imag.T) @ C.astype(np.float64)
    j = np.arange(nlag * 128, dtype=np.float64)
    P = np.exp(np.outer(j, lnl))                      # lam^j, (W, N) complex128
    H = P.real @ A_re - P.imag @ A_im                 # (W, M)
    H[0] += D[0].astype(np.float64)

    def flip_tiles(Hm, ntile):
        Hf = Hm.reshape(ntile, 128, m)[:, ::-1, :]
        return np.ascontiguousarray(Hf.reshape(ntile * 128, m)).astype(np.float32)

    dorep = np.ascontiguousarray(np.broadcast_to(Do.astype(np.float32), (128, m)))
    if mode == "f32":
        return {"ht": flip_tiles(H, nlag), "dorep": dorep}
    H_hi = _round_mant(H)
    if mode == "f32r1":
        return {"ht": flip_tiles(H_hi, nlag), "dorep": dorep}
    nlo = nlag if mode == "f32r3" else 1
    H_lo = _round_mant(H[: nlo * 128] - H_hi[: nlo * 128])
    return {
        "ht": flip_tiles(H_hi, nlag),
        "htlo": flip_tiles(H_lo, nlo),
        "dorep": dorep,
    }


def make_in_maps(x, weights, t=T, nlag=NLAG, bloc=BLOC, ncores=NCORES, mode=MODE):
    rpad, _, _ = _dims(t, nlag, bloc)
    lo_np = np.float32
    x64 = x[:, :, 0].astype(np.float64)
    if mode == "f32":
        x_hi, x_lo = x64, None
    else:
        x_hi = _round_mant(x64)
        x_lo = _round_mant(x64 - x_hi) if mode in ("f32r_hybrid", "f32r3") else None
    in_maps = []
    for c in range(ncores):
        sl = slice(c * bloc, (c + 1) * bloc)
        xpad = np.zeros((rpad + t, bloc), np.float32)
        xpad[rpad:, :] = x_hi[sl].T
        im = dict(weights)
        im["xpad"] = xpad
        if x_lo is not None:
            xpadlo = np.zeros((rpad + t, bloc), lo_np)
            xpadlo[rpad:, :] = x_lo[sl].T.astype(lo_np)
            im["xpadlo"] = xpadlo
        in_maps.append(im)
    return in_maps


_prog_cache = {}


def kernel(x, lnl_re, lnl_im, W_r, W_i, C, D, Do):
    from concourse.bass_utils import run_bass_kernel_spmd

    # coerce to numpy (host math needs real float64; jax arrays stay fp32)
    x = np.asarray(x)
    lnl_re, lnl_im = np.asarray(lnl_re), np.asarray(lnl_im)
    W_r, W_i = np.asarray(W_r), np.asarray(W_i)
    C, D, Do = np.asarray(C), np.asarray(D), np.asarray(Do)

    key = (NLAG, MODE)
    if key not in _prog_cache:
        _prog_cache[key] = build_program()
    nc = _prog_cache[key]

    weights = host_weights(lnl_re, lnl_im, W_r, W_i, C, D, Do)
    in_maps = make_in_maps(np.asarray(x, np.float32), weights)
    res = run_bass_kernel_spmd(nc, in_maps, core_ids=list(range(NCORES)))
    y = np.concatenate([res.results[i]["y"] for i in range(NCORES)], axis=0)
    return np.ascontiguousarray(y.astype(np.float32))
